# revision 12
# baseline (speedup 1.0000x reference)
"""DIN-style attention layer on 8 Trainium2 NeuronCores.

Problem: q[B,64], k[B,200,64], v[B,200,64], mask[B,200]; per-token MLP on
DIN features concat([q,k,q-k,q*k]) -> 80 -> 40 -> 1 logits, masked softmax
over T, then attn-weighted sum of v. B=2048 sharded over 8 cores.

Math refactor (host):
  info@W1 = q@(W1a+W1c) + k@(W1b-W1c) + (q*k)@W1d   with W1=[W1a;W1b;W1c;W1d]
  => h1_b = relu( Wb_eff^T kt_b + beta_b ),  Wb_eff = (W1b-W1c) + q_b*W1d
     beta_b = q_b@(W1a+W1c) + b1   (folded in as a 65th all-ones row of kt)
bf is dropped: softmax is shift-invariant. mask applied additively (-1e9).

Device (per core, 256 batches = 128 pairs = 16 groups of 8 pairs):
  L1 per batch:  psum[80,400]  = w1b[65,80]^T @ kt1[65,200]   (pair-packed N)
  relu (ACT) -> h1 bf16 [80,400]
  L2 per pair:   psum[104,400] rows {0-39,64-103} = W2^T @ h1  (2 pairs/bank)
  relu+b2 (DVE tensor_scalar add,max) -> h2 bf16 [104,400]
  L3 per 2 pairs: blockdiag Wf [104,2] -> logits psum rows {32j,32j+1}
  -> sparse-16-row [128,400] logits tile; +mask, softmax (DVE/ACT, accum_out)
  -> PE-transpose attn -> attn^T; out = v^T @ attn^T per pair (v stationary)
"""

import os
import sys

import numpy as np

for _p in ("/opt/trn_rl_repo", "/root/.axon_site/_ro/trn_rl_repo"):
    if os.path.isdir(_p) and _p not in sys.path:
        sys.path.insert(0, _p)

import ml_dtypes

BF16 = ml_dtypes.bfloat16

B, T, D = 2048, 200, 64
H1, H2 = 80, 40
NCORES = 8
BC = B // NCORES          # 256 batches per core
PAIRS = BC // 2           # 128
NG = PAIRS // 8           # 16 groups of 8 pairs (16 batches)


def _build_bass():
    from concourse import bass, bacc, tile
    from concourse import mybir

    dt = mybir.dt
    nc = bacc.Bacc("TRN2", target_bir_lowering=False, debug=False)

    kt1 = nc.declare_dram_parameter("kt1", [D + 1, BC, T], dt.bfloat16, False)
    w1b = nc.declare_dram_parameter("w1b", [D + 1, BC, H1], dt.bfloat16, False)
    v2a = nc.declare_dram_parameter("v2a", [128, PAIRS, 128], dt.bfloat16, False)
    v2b = nc.declare_dram_parameter("v2b", [72, PAIRS, 128], dt.bfloat16, False)
    amask = nc.declare_dram_parameter("amask", [NG, 128, 2 * T], dt.bfloat16, False)
    w2 = nc.declare_dram_parameter("w2", [H1, 64], dt.bfloat16, False)
    wfbd = nc.declare_dram_parameter("wfbd", [128, 32], dt.bfloat16, False)
    b2s = nc.declare_dram_parameter("b2s", [128, 1], dt.float32, False)
    ident = nc.declare_dram_parameter("ident", [128, 128], dt.bfloat16, False)
    outp = nc.declare_dram_parameter("outp", [128, PAIRS, 2], dt.float32, True)
    osum = nc.declare_dram_parameter("osum", [NG, 128, 2], dt.float32, True)

    GB = 16  # batches per group
    W2T = 2 * T  # 400

    with tile.TileContext(nc) as tc:
        with (
            tc.tile_pool(name="consts", bufs=1) as cpool,
            tc.tile_pool(name="kin", bufs=3) as kpool,
            tc.tile_pool(name="win", bufs=3) as wpool,
            tc.tile_pool(name="vin", bufs=3) as vpool,
            tc.tile_pool(name="min", bufs=3) as mpool,
            tc.tile_pool(name="h1", bufs=6) as h1pool,
            tc.tile_pool(name="h2", bufs=4) as h2pool,
            tc.tile_pool(name="sm", bufs=3) as smpool,
            tc.tile_pool(name="small", bufs=4) as spool,
            tc.tile_pool(name="ats", bufs=2) as atspool,
            tc.tile_pool(name="outs", bufs=2) as opool,
            tc.tile_pool(name="ph1", bufs=3, space="PSUM") as ph1pool,
            tc.tile_pool(name="ph2", bufs=2, space="PSUM") as ph2pool,
            tc.tile_pool(name="plg", bufs=1, space="PSUM") as plgpool,
            tc.tile_pool(name="pt", bufs=1, space="PSUM") as ptpool,
            tc.tile_pool(name="op", bufs=1, space="PSUM") as oppool,
        ):
            w2_t = cpool.tile([H1, 64], dt.bfloat16)
            nc.sync.dma_start(w2_t[:], w2[:])
            wfbd_t = cpool.tile([128, 32], dt.bfloat16)
            nc.sync.dma_start(wfbd_t[:], wfbd[:])
            b2s_t = cpool.tile([128, 1], dt.float32)
            nc.sync.dma_start(b2s_t[:], b2s[:])
            id_t = cpool.tile([128, 128], dt.bfloat16)
            nc.sync.dma_start(id_t[:], ident[:])
            # Warm the DVE vector clock past the const DMAs: TensorScalarPtr
            # (h2-relu with AP scalar) only has one sync-wait slot, so it must
            # not be the first DVE op to observe the b2s DMA completion.
            dve_warm = cpool.tile([128, 1], dt.float32)
            nc.vector.tensor_copy(dve_warm[:], b2s_t[:])
            zero_t = cpool.tile([128, W2T], dt.bfloat16)
            nc.vector.memset(zero_t[:], 0.0)

            for g in range(NG):
                KT = kpool.tile([D + 1, GB, T], dt.bfloat16)
                nc.sync.dma_start(KT[:], kt1[:, g * GB : (g + 1) * GB, :])
                WB = wpool.tile([D + 1, GB, H1], dt.bfloat16)
                nc.sync.dma_start(WB[:], w1b[:, g * GB : (g + 1) * GB, :])
                V2A = vpool.tile([128, 8, 128], dt.bfloat16, name="V2A")
                nc.sync.dma_start(V2A[:], v2a[:, g * 8 : (g + 1) * 8, :])
                V2B = vpool.tile([72, 8, 128], dt.bfloat16, name="V2B")
                nc.sync.dma_start(V2B[:], v2b[:, g * 8 : (g + 1) * 8, :])
                AM = mpool.tile([128, W2T], dt.bfloat16)
                nc.sync.dma_start(AM[:], amask[g])

                PLG = plgpool.tile([128, W2T], dt.float32)
                H2Ss = {}

                def l3(jj):
                    nc.tensor.matmul(
                        PLG[32 * jj : 32 * jj + 32, :],
                        lhsT=wfbd_t[:],
                        rhs=H2Ss[jj][:],
                        start=True,
                        stop=True,
                        tile_position=(0, 32 * jj),
                    )

                for jj in range(4):  # 2-pair chunks
                    PH2 = ph2pool.tile([128, W2T], dt.float32)
                    H2S = h2pool.tile([128, W2T], dt.bfloat16)
                    for i in range(2):  # pair within chunk
                        PH1 = ph1pool.tile([H1, W2T], dt.float32)
                        for jb in range(2):  # batch within pair
                            bi = 4 * jj + 2 * i + jb
                            nc.tensor.matmul(
                                PH1[:, jb * T : (jb + 1) * T],
                                lhsT=WB[:, bi, :],
                                rhs=KT[:, bi, :],
                                start=True,
                                stop=True,
                            )
                        H1S = h1pool.tile([H1, W2T], dt.bfloat16)
                        if i == 0:
                            nc.vector.tensor_tensor(
                                out=H1S[:],
                                in0=PH1[:],
                                in1=zero_t[0:H1, :],
                                op=mybir.AluOpType.max,
                            )
                        else:
                            nc.scalar.activation(
                                H1S[:],
                                PH1[:],
                                mybir.ActivationFunctionType.Relu,
                            )
                        if i == 0 and jj > 0:
                            l3(jj - 1)
                        nc.tensor.matmul(
                            PH2[64 * i : 64 * i + 64, :],
                            lhsT=w2_t[:],
                            rhs=H1S[:],
                            start=True,
                            stop=True,
                            tile_position=(0, 64 * i),
                        )
                    nc.scalar.activation(
                        H2S[:],
                        PH2[:],
                        mybir.ActivationFunctionType.Relu,
                        bias=b2s_t[:],
                    )
                    H2Ss[jj] = H2S
                l3(3)

                # masked unnormalized softmax: logits are tiny, masked
                # entries are -1e9 (exp -> 0), so no max-subtraction needed;
                # per-batch normalization happens on the host via osum.
                LM = smpool.tile([128, W2T], dt.float32)
                nc.vector.tensor_add(LM[:], PLG[:], AM[:])
                EX = smpool.tile([128, 2, T], dt.bfloat16)
                nc.scalar.activation(
                    EX[:],
                    LM[:],
                    mybir.ActivationFunctionType.Exp,
                )
                SUM = spool.tile([128, 2], dt.float32)
                nc.vector.tensor_reduce(
                    out=SUM[:],
                    in_=EX[:],
                    axis=mybir.AxisListType.X,
                    op=mybir.AluOpType.add,
                )
                nc.sync.dma_start(osum[g], SUM[:])

                # transpose exp-weights: [sparse-b, t] -> [t, sparse-b]
                PT = ptpool.tile([128, 512], dt.bfloat16)
                nc.tensor.transpose(PT[0:128, 0:128], EX[:, 0, 0:128], id_t[:])
                nc.tensor.transpose(PT[0:72, 128:256], EX[:, 0, 128:200], id_t[:])
                nc.tensor.transpose(PT[0:128, 256:384], EX[:, 1, 0:128], id_t[:])
                nc.tensor.transpose(PT[0:72, 384:512], EX[:, 1, 128:200], id_t[:])
                ATS = atspool.tile([128, 2, 256], dt.bfloat16)
                nc.vector.tensor_copy(ATS[:, 0, 0:128], PT[:, 0:128])
                nc.vector.tensor_copy(ATS[0:72, 0, 128:256], PT[0:72, 128:256])
                nc.vector.tensor_copy(ATS[:, 1, 0:128], PT[:, 256:384])
                nc.vector.tensor_copy(ATS[0:72, 1, 128:256], PT[0:72, 384:512])

                # out = v^T @ attn^T per pair (v stationary)
                OPT = oppool.tile([128, 8, 2], dt.float32)
                for q in range(8):
                    jj, i = q // 2, q % 2
                    c = 32 * jj + i
                    nc.tensor.matmul(
                        OPT[:, q, :],
                        lhsT=V2A[:, q, :],
                        rhs=ATS[0:128, :, c],
                        start=True,
                        stop=False,
                    )
                    nc.tensor.matmul(
                        OPT[:, q, :],
                        lhsT=V2B[:, q, :],
                        rhs=ATS[0:72, :, 128 + c],
                        start=False,
                        stop=True,
                    )
                OUTS = opool.tile([128, 8, 2], dt.float32)
                nc.vector.tensor_copy(OUTS[:], OPT[:])
                nc.sync.dma_start(outp[:, g * 8 : (g + 1) * 8, :], OUTS[:])

    nc.compile()
    return nc


_NC_CACHE = {}


def _get_nc():
    if "nc" not in _NC_CACHE:
        _NC_CACHE["nc"] = _build_bass()
    return _NC_CACHE["nc"]


def _prep_core(qc, kc, vc, mc, W1, b1, W2, b2, Wf):
    """Build the per-core DRAM input dict (numpy, host-side)."""
    f32 = np.float32
    W1a, W1b_, W1c, W1d = W1[0:64], W1[64:128], W1[128:192], W1[192:256]

    # kt1 [65, BC, T]: rows 0-63 = k^T per batch, row 64 = ones
    kt1 = np.empty((D + 1, BC, T), dtype=BF16)
    kt1[0:D] = kc.transpose(2, 0, 1).astype(BF16)
    kt1[D] = np.ones((BC, T), dtype=BF16)

    # w1b [65, BC, H1]: rows 0-63 = (W1b-W1c) + q_b*W1d, row 64 = beta_b
    wb_eff = (W1b_ - W1c)[None, :, :] + qc[:, :, None] * W1d[None, :, :]
    beta = qc @ (W1a + W1c) + b1[None, :]
    w1b = np.empty((D + 1, BC, H1), dtype=BF16)
    w1b[0:D] = wb_eff.transpose(1, 0, 2).astype(BF16)
    w1b[D] = beta.astype(BF16)

    # v2a/v2b: [t', pair, batch-in-pair*64+d] for t-chunks 0:128 / 128:200
    vpad = np.zeros((PAIRS, 2, 256, D), dtype=f32)
    vpad[:, :, 0:T] = vc.reshape(PAIRS, 2, T, D)
    vfull = np.ascontiguousarray(
        vpad.reshape(PAIRS, 2, 2, 128, D).transpose(3, 0, 2, 1, 4).reshape(
            128, PAIRS, 2, 128
        )
    ).astype(BF16)
    v2a = np.ascontiguousarray(vfull[:, :, 0, :])
    v2b = np.ascontiguousarray(vfull[0:72, :, 1, :])

    # amask [NG, 128, 400] sparse-16 rows {32jj+i}, cols jb*200
    m4 = np.where(mc == 0, f32(-1e9), f32(0.0)).astype(f32)
    m5 = m4.reshape(NG, 4, 2, 2, T)  # [g, jj, i, jb, t]
    am_f = np.zeros((NG, 128, 2 * T), dtype=f32)
    for jj in range(4):
        for i in range(2):
            for jb in range(2):
                am_f[:, 32 * jj + i, jb * T : (jb + 1) * T] = m5[:, jj, i, jb]
    am = am_f.astype(BF16)

    wfbd = np.zeros((128, 32), dtype=BF16)
    wfbd[0:H2, 0] = Wf[:, 0].astype(BF16)
    wfbd[64 : 64 + H2, 1] = Wf[:, 0].astype(BF16)
    b2s = np.zeros((128, 1), dtype=f32)
    b2s[0:H2, 0] = b2
    b2s[64 : 64 + H2, 0] = b2
    w2p = np.zeros((H1, 64), dtype=BF16)
    w2p[:, 0:H2] = W2.astype(BF16)

    return {
        "kt1": kt1,
        "w1b": w1b,
        "v2a": v2a,
        "v2b": v2b,
        "amask": am,
        "w2": w2p,
        "wfbd": wfbd,
        "b2s": b2s,
        "ident": np.eye(128, dtype=BF16),
    }


def _postprocess(res_c):
    """[128,PAIRS,2] unnormalized sums + [NG,128,2] exp-sums -> [BC, D]."""
    op = np.asarray(res_c["outp"], dtype=np.float32)
    osum = np.asarray(res_c["osum"], dtype=np.float32)
    oc = np.empty((BC, D), dtype=np.float32)
    # batch 16g+4jj+2i+jb -> osum[g, 32jj+i, jb]
    s = np.empty(BC, dtype=np.float32)
    for g in range(NG):
        for jj in range(4):
            for i in range(2):
                for jb in range(2):
                    s[16 * g + 4 * jj + 2 * i + jb] = osum[g, 32 * jj + i, jb]
    s = np.where(s == 0.0, np.float32(1.0), s)
    oc[0::2, :] = op[0:D, :, 0].T / s[0::2][:, None]
    oc[1::2, :] = op[D : 2 * D, :, 1].T / s[1::2][:, None]
    return oc


def kernel(q, k, v, mask, W1, b1, W2, b2, Wf, bf, **_):
    from concourse.bass_utils import run_bass_kernel_spmd

    q = np.asarray(q, dtype=np.float32)
    k = np.asarray(k, dtype=np.float32)
    v = np.asarray(v, dtype=np.float32)
    mask = np.asarray(mask)
    W1 = np.asarray(W1, dtype=np.float32)
    b1 = np.asarray(b1, dtype=np.float32)
    W2 = np.asarray(W2, dtype=np.float32)
    b2 = np.asarray(b2, dtype=np.float32)
    Wf = np.asarray(Wf, dtype=np.float32)

    nc = _get_nc()
    in_maps = []
    for c in range(NCORES):
        s = slice(c * BC, (c + 1) * BC)
        in_maps.append(_prep_core(q[s], k[s], v[s], mask[s], W1, b1, W2, b2, Wf))

    res = run_bass_kernel_spmd(nc, in_maps, list(range(NCORES)))
    results = res.results

    out = np.empty((B, D), dtype=np.float32)
    for c in range(NCORES):
        out[c * BC : (c + 1) * BC] = _postprocess(results[c])
    return out


if __name__ == "__main__":
    rng = np.random.default_rng(0)
    inputs = {
        "q": rng.standard_normal((B, D), dtype=np.float32),
        "k": rng.standard_normal((B, T, D), dtype=np.float32),
        "v": rng.standard_normal((B, T, D), dtype=np.float32),
        "mask": rng.integers(0, 2, size=(B, T)).astype(np.int32),
        "W1": rng.standard_normal((4 * D, H1), dtype=np.float32) * 0.05,
        "b1": np.zeros(H1, np.float32),
        "W2": rng.standard_normal((H1, H2), dtype=np.float32) * 0.05,
        "b2": np.zeros(H2, np.float32),
        "Wf": rng.standard_normal((H2, 1), dtype=np.float32) * 0.05,
        "bf": np.zeros(1, np.float32),
    }
    out = kernel(**inputs)
    print(out.shape, out.dtype, np.abs(out).max())

